# revision 1
# baseline (speedup 1.0000x reference)
"""TRN2 Bass kernel for the equivariant GNN message-passing layer.

Architecture (per core):
  - Host sorts edges by dst, packs into "pairs" (256 edge slots, <=16 dst
    nodes), 8 pairs per block (128 out rows / block), blocks split across
    8 cores.
  - Device, per block b:
      gather   x_all[128, 16, 128] = features[srcw[b]]          (indirect DMA)
      per tile t (16 per block):
        Ocat[e,(j,d)] = (iota_d == dstl[e]) * sh[e,j]           (1 DVE op)
        G[f,(j,d)]   += x_t^T @ Ocat   (pair-accumulated PSUM)  (PE)
      per pair: stage G into SBUF at pair offset                (ScalarE)
      finalize: po[s,o] = sum_j G_j^T @ T_j  (9 chained MMs)    (PE)
      write out[b*128:(b+1)*128] = po                           (DMA)
  - Host inverse-permutes device rows to global node ids. No collectives.

msg[e,o] = sum_j sh[e,j] * sum_f x[e,f] * T[j][f,o], T = CG (x) W combined.
"""
import math
import numpy as np

import concourse.bass as bass
import concourse.tile as tile
from concourse import bacc, mybir
from concourse.bass_utils import run_bass_kernel_spmd

P = 128
WINDOW = 16                # dst-node slots per pair
TILES_PER_PAIR = 2
PAIR_EDGES = P * TILES_PER_PAIR       # 256
PAIRS_PER_BLOCK = 8
TILES_PER_BLOCK = TILES_PER_PAIR * PAIRS_PER_BLOCK  # 16
BLOCK_EDGES = PAIR_EDGES * PAIRS_PER_BLOCK          # 2048
N_CORES = 8
NUM_NODES = 50000
NUM_EDGES = 800000
FDIM = 128
SHDIM = 9
OCW = SHDIM * WINDOW       # 144  Ocat width, (j,d) j-major

# ---------------- irreps / CG (self-contained copy of reference config) ----
IRREPS_IN = [(32, 0), (32, 1)]
IRREPS_SH = [(1, 0), (1, 1), (1, 2)]
IRREPS_OUT = [(32, 0), (32, 1)]
PATHS = [(0, 0, 0), (0, 1, 1), (1, 0, 1), (1, 1, 0), (1, 2, 1)]
N_PATHS_TO_OUT = {io: sum(1 for p in PATHS if p[2] == io) for io in range(len(IRREPS_OUT))}


def _fact(n):
    return math.factorial(int(round(n)))


def _su2_cg(j1, j2, j3):
    C = np.zeros((2 * j1 + 1, 2 * j2 + 1, 2 * j3 + 1))
    for m1 in range(-j1, j1 + 1):
        for m2 in range(-j2, j2 + 1):
            m3 = m1 + m2
            if abs(m3) > j3:
                continue
            vmin = int(max(-j1 + j2 + m3, -j1 + m1, 0))
            vmax = int(min(j2 + j3 + m1, j3 - j1 + j2, j3 + m3))
            c = math.sqrt((2 * j3 + 1) * _fact(j3 + j1 - j2) * _fact(j3 - j1 + j2)
                          * _fact(j1 + j2 - j3) / _fact(j1 + j2 + j3 + 1))
            c *= math.sqrt(_fact(j3 + m3) * _fact(j3 - m3) * _fact(j1 - m1)
                           * _fact(j1 + m1) * _fact(j2 - m2) * _fact(j2 + m2))
            s = 0.0
            for v in range(vmin, vmax + 1):
                s += ((-1.0) ** (v + j2 + m2) * _fact(j2 + j3 + m1 - v) * _fact(j1 - m1 + v)
                      / (_fact(v) * _fact(j3 - j1 + j2 - v) * _fact(j3 + m3 - v)
                         * _fact(v + j1 - j2 - m3)))
            C[j1 + m1, j2 + m2, j3 + m3] = c * s
    return C


def _q(l):
    q = np.zeros((2 * l + 1, 2 * l + 1), dtype=np.complex128)
    for m in range(-l, 0):
        q[l + m, l + abs(m)] = 1.0 / np.sqrt(2.0)
        q[l + m, l - abs(m)] = -1j / np.sqrt(2.0)
    q[l, l] = 1.0
    for m in range(1, l + 1):
        q[l + m, l + abs(m)] = (-1.0) ** m / np.sqrt(2.0)
        q[l + m, l - abs(m)] = 1j * (-1.0) ** m / np.sqrt(2.0)
    return (-1j) ** l * q


def _real_cg(l1, l2, l3):
    C = _su2_cg(l1, l2, l3).astype(np.complex128)
    C = np.einsum('ij,kl,mn,ikn->jlm', _q(l1), _q(l2), np.conj(_q(l3).T), C)
    C = np.real(C)
    return (C / np.linalg.norm(C)).astype(np.float32)


_CG = [_real_cg(IRREPS_IN[i1][1], IRREPS_SH[i2][1], IRREPS_OUT[io][1])
       for (i1, i2, io) in PATHS]


def make_T(weights: np.ndarray) -> np.ndarray:
    """T [9, 128, 128]: msg[e,o] = sum_j sh[e,j] sum_f x[e,f] T[j,f,o]."""
    in_off = [0, 32]
    out_off = [0, 32]
    sh_off = [0, 1, 4]
    T = np.zeros((SHDIM, FDIM, FDIM), dtype=np.float64)
    for p, (i1, i2, io) in enumerate(PATHS):
        u, v = IRREPS_IN[i1][0], IRREPS_SH[i2][0]
        alpha = 1.0 / math.sqrt(u * v * N_PATHS_TO_OUT[io])
        cg = np.asarray(_CG[p], dtype=np.float64)
        l1d, l2d, l3d = cg.shape
        W = np.asarray(weights[p], dtype=np.float64)
        contrib = alpha * np.einsum("uw,ijk->juiwk", W, cg)  # [j,u,i,w,k]
        jdim, udim, idim, wdim, kdim = contrib.shape
        T[sh_off[i2]:sh_off[i2] + jdim,
          in_off[i1]:in_off[i1] + udim * idim,
          out_off[io]:out_off[io] + wdim * kdim] += contrib.reshape(
              jdim, udim * idim, wdim * kdim)
    return T.astype(np.float32)


# ---------------- host packing ----------------

def pack_edges(src, dst, sh):
    """Pack edges into pairs of pure-parity tiles.

    Pair = (even-src tile [<=128 slots], odd-src tile [<=128 slots]),
    window <= 16 dst nodes. Per-pair slot arrays are [2, 128]
    (parity, slot). Returns arrays + win_ids + blocks_per_core.
    """
    order = np.argsort(dst, kind="stable")
    s_src, s_dst, s_sh = src[order], dst[order], sh[order]

    uniq, counts = np.unique(s_dst, return_counts=True)
    starts = np.concatenate([[0], np.cumsum(counts)]).astype(np.int64)
    n_uniq = len(uniq)

    # per-node even/odd edge counts (edges sorted by dst; parity of src)
    par = (s_src & 1).astype(np.int64)
    cum_par = np.concatenate([[0], np.cumsum(par)])
    odd_counts = cum_par[starts[1:]] - cum_par[starts[:-1]]
    even_counts = counts - odd_counts

    pair_of_node = np.zeros(n_uniq, dtype=np.int64)
    slot_of_node = np.zeros(n_uniq, dtype=np.int64)
    pi = 0
    cur_e = 0
    cur_o = 0
    cur_nodes = 0
    for ni in range(n_uniq):
        ce, co = even_counts[ni], odd_counts[ni]
        assert ce <= P and co <= P, "node parity degree exceeds tile"
        if cur_e + ce > P or cur_o + co > P or cur_nodes >= WINDOW:
            pi += 1
            cur_e = cur_o = cur_nodes = 0
        pair_of_node[ni] = pi
        slot_of_node[ni] = cur_nodes
        cur_e += ce
        cur_o += co
        cur_nodes += 1
    n_pairs = pi + 1

    n_blocks = -(-n_pairs // PAIRS_PER_BLOCK)
    blocks_per_core = -(-n_blocks // N_CORES)
    n_pairs_pad = blocks_per_core * N_CORES * PAIRS_PER_BLOCK

    # per-edge positions: within (pair, parity), edges take consecutive slots
    node_of_edge = np.searchsorted(uniq, s_dst)
    edge_pair = pair_of_node[node_of_edge]
    # position within (pair, parity) via cumulative counting
    key = edge_pair * 2 + par                     # group id per edge
    # edges sorted by dst (hence by pair) but parities interleave; stable order
    ordpp = np.argsort(key, kind="stable")
    inv = np.empty_like(ordpp)
    inv[ordpp] = np.arange(len(ordpp))
    ks = key[ordpp]
    kchange = np.concatenate([[True], ks[1:] != ks[:-1]])
    gstart = np.where(kchange)[0]
    gstart_of = gstart[np.cumsum(kchange) - 1]
    pos_sorted = np.arange(len(ks)) - gstart_of
    edge_pos = pos_sorted[inv]                    # slot within (pair, parity)

    src_idx = np.zeros((n_pairs_pad, 2, P), dtype=np.int32)
    sh_arr = np.zeros((n_pairs_pad, 2, P, SHDIM), dtype=np.float32)
    dstl = np.zeros((n_pairs_pad, 2, P), dtype=np.float32)
    win_ids = -np.ones((n_pairs_pad, WINDOW), dtype=np.int64)

    src_idx[edge_pair, par, edge_pos] = s_src >> 1
    sh_arr[edge_pair, par, edge_pos] = s_sh
    dstl[edge_pair, par, edge_pos] = slot_of_node[node_of_edge].astype(np.float32)
    win_ids[pair_of_node, slot_of_node] = uniq

    return src_idx, sh_arr, dstl, win_ids, blocks_per_core, n_pairs


# ---------------- device program ----------------

_prog_cache = {}


def build_program(Bc: int):
    """Build + compile the SPMD Bass program for Bc blocks per core.

    Tile order per block: t in [0,8) = pair t's even-src tile,
    t in [8,16) = pair (t-8)'s odd-src tile. Features split by src parity
    into two tables so dma_gather's int16 indices (src>>1 < 25000) fit.
    """
    if Bc in _prog_cache:
        return _prog_cache[Bc]
    dt = mybir.dt
    NH = NUM_NODES // 2
    GIDX = PAIRS_PER_BLOCK * P // 16     # 64 idx cols per gather
    nc = bacc.Bacc("TRN2", target_bir_lowering=False, debug=False,
                   num_devices=N_CORES, num_swdge_queues=4)
    fe = nc.dram_tensor("fe", [NH, FDIM], dt.float32, kind="ExternalInput").ap()
    fo = nc.dram_tensor("fo", [NH, FDIM], dt.float32, kind="ExternalInput").ap()
    idxe = nc.dram_tensor("idxe", [Bc, P, GIDX], dt.int16, kind="ExternalInput").ap()
    idxo = nc.dram_tensor("idxo", [Bc, P, GIDX], dt.int16, kind="ExternalInput").ap()
    shw = nc.dram_tensor("shw", [Bc, P, TILES_PER_BLOCK * SHDIM], dt.float32, kind="ExternalInput").ap()
    dstw = nc.dram_tensor("dstw", [Bc, P, TILES_PER_BLOCK], dt.float32, kind="ExternalInput").ap()
    iot = nc.dram_tensor("iot", [P, OCW], dt.float32, kind="ExternalInput").ap()
    tmat = nc.dram_tensor("tmat", [P, SHDIM * FDIM], dt.float32, kind="ExternalInput").ap()
    out = nc.dram_tensor("out", [Bc * P, FDIM], dt.float32, kind="ExternalOutput").ap()

    HBLK = PAIRS_PER_BLOCK * FDIM        # 1024 cols per parity section

    with tile.TileContext(nc) as tc:
        with (
            tc.tile_pool(name="const", bufs=1) as cpool,
            tc.tile_pool(name="xin", bufs=3) as xpool,
            tc.tile_pool(name="edgein", bufs=3) as epool,
            tc.tile_pool(name="oc", bufs=4) as ocpool,
            tc.tile_pool(name="gps", bufs=2, space="PSUM") as gpool,
            tc.tile_pool(name="stag", bufs=2) as spool,
            tc.tile_pool(name="pops", bufs=2, space="PSUM") as popool,
            tc.tile_pool(name="osb", bufs=2) as opool,
        ):
            iot_sb = cpool.tile([P, OCW], dt.float32)
            nc.sync.dma_start(out=iot_sb[:], in_=iot[:, :])
            t_sb = cpool.tile([P, SHDIM * FDIM], dt.float32)
            nc.sync.dma_start(out=t_sb[:], in_=tmat[:, :])

            for b in range(Bc):
                ie_sb = epool.tile([P, GIDX], dt.int16, tag="ie")
                nc.sync.dma_start(out=ie_sb[:], in_=idxe[b, :, :])
                io_sb = epool.tile([P, GIDX], dt.int16, tag="io")
                nc.sync.dma_start(out=io_sb[:], in_=idxo[b, :, :])
                sh_sb = epool.tile([P, TILES_PER_BLOCK * SHDIM], dt.float32, tag="sh")
                nc.sync.dma_start(out=sh_sb[:], in_=shw[b, :, :])
                dst_sb = epool.tile([P, TILES_PER_BLOCK], dt.float32, tag="dst")
                nc.sync.dma_start(out=dst_sb[:], in_=dstw[b, :, :])

                x_all = xpool.tile([P, TILES_PER_BLOCK * FDIM], dt.float32)
                nc.gpsimd.dma_gather(
                    out_ap=x_all[:, :HBLK].rearrange("p (t f) -> p t f", f=FDIM),
                    in_ap=fe[:, :], idxs_ap=ie_sb[:],
                    num_idxs=PAIRS_PER_BLOCK * P,
                    num_idxs_reg=PAIRS_PER_BLOCK * P,
                    elem_size=FDIM, single_packet=False,
                    queue_num=(2 * b) % 4,
                )
                nc.gpsimd.dma_gather(
                    out_ap=x_all[:, HBLK:].rearrange("p (t f) -> p t f", f=FDIM),
                    in_ap=fo[:, :], idxs_ap=io_sb[:],
                    num_idxs=PAIRS_PER_BLOCK * P,
                    num_idxs_reg=PAIRS_PER_BLOCK * P,
                    elem_size=FDIM, single_packet=False,
                    queue_num=(2 * b + 1) % 4,
                )

                stag = spool.tile([P, SHDIM * P], dt.float32)
                for pair in range(PAIRS_PER_BLOCK):
                    g_ps = gpool.tile([P, OCW], dt.float32, space="PSUM")
                    for u in range(TILES_PER_PAIR):
                        t = u * PAIRS_PER_BLOCK + pair   # even tile, then odd tile
                        oc = ocpool.tile([P, OCW], dt.float32)
                        nc.vector.scalar_tensor_tensor(
                            out=oc[:].rearrange("p (j d) -> p j d", j=SHDIM),
                            in0=iot_sb[:].rearrange("p (j d) -> p j d", j=SHDIM),
                            scalar=dst_sb[:, t:t + 1],
                            in1=sh_sb[:, t * SHDIM:(t + 1) * SHDIM]
                                .to_broadcast([P, SHDIM, WINDOW]),
                            op0=mybir.AluOpType.is_equal,
                            op1=mybir.AluOpType.mult,
                        )
                        nc.tensor.matmul(
                            out=g_ps[:],
                            lhsT=x_all[:, t * FDIM:(t + 1) * FDIM],
                            rhs=oc[:],
                            start=(u == 0),
                            stop=(u == TILES_PER_PAIR - 1),
                        )
                    # stage: stag[:, j*128 + pair*16 + d] = g_ps[:, j*16+d]
                    nc.scalar.copy(
                        out=stag[:].rearrange("p (j g) -> p j g", j=SHDIM)
                            [:, :, pair * WINDOW:(pair + 1) * WINDOW],
                        in_=g_ps[:].rearrange("p (j d) -> p j d", j=SHDIM),
                    )

                po = popool.tile([P, FDIM], dt.float32, space="PSUM")
                for j in range(SHDIM):
                    nc.tensor.matmul(
                        out=po[:],
                        lhsT=stag[:, j * P:(j + 1) * P],
                        rhs=t_sb[:, j * FDIM:(j + 1) * FDIM],
                        start=(j == 0),
                        stop=(j == SHDIM - 1),
                    )
                o_sb = opool.tile([P, FDIM], dt.float32)
                nc.vector.tensor_copy(out=o_sb[:], in_=po[:])
                nc.sync.dma_start(out=out[b * P:(b + 1) * P, :], in_=o_sb[:])

    nc.compile()
    _prog_cache[Bc] = nc
    return nc


# ---------------- full kernel ----------------

def prepare(features, edge_sh, edge_index, weights):
    """Host-side sharding: returns (Bc, in_maps, win_ids)."""
    features = np.asarray(features, dtype=np.float32)
    edge_sh = np.asarray(edge_sh, dtype=np.float32)
    edge_index = np.asarray(edge_index, dtype=np.int32)
    weights = np.asarray(weights, dtype=np.float32)

    T = make_T(weights)                       # [9,128,128]
    tmat_host = np.ascontiguousarray(T.transpose(1, 0, 2).reshape(FDIM, SHDIM * FDIM))
    iot_host = np.tile(np.arange(WINDOW, dtype=np.float32), SHDIM)[None, :].repeat(P, 0)
    iot_host = np.ascontiguousarray(iot_host)

    src, dst = edge_index[0], edge_index[1]
    src_idx, sh_arr, dstl, win_ids, Bc, n_pairs = pack_edges(src, dst, edge_sh)
    # per-pair arrays are [npp, 2(par), P, ...]; device tile t = par*8 + pair
    npp = src_idx.shape[0]

    fe = np.ascontiguousarray(features[0::2])
    fo = np.ascontiguousarray(features[1::2])

    def to_dev(a, dtype):
        extra = a.shape[3:]
        a = a.reshape(N_CORES, Bc, PAIRS_PER_BLOCK, 2, P, *extra)
        a = np.moveaxis(a, 4, 2)                   # [core,Bc,P,pair,par,...]
        a = np.swapaxes(a, 3, 4)                   # [core,Bc,P,par,pair,...]
        a = a.reshape(N_CORES, Bc, P, TILES_PER_BLOCK, *extra)
        return np.ascontiguousarray(a.astype(dtype))

    dstw_h = to_dev(dstl, np.float32)                      # [8,Bc,128,16]
    shw_h = to_dev(sh_arr, np.float32).reshape(N_CORES, Bc, P, TILES_PER_BLOCK * SHDIM)

    # gather idx arrays: per (core, block, parity): flat i = pair*128 + slot,
    # wrapped to [128, 64]: row 16k + i%16 (replicated k=0..7), col i//16
    idx_flat = src_idx.reshape(N_CORES, Bc, PAIRS_PER_BLOCK, 2, P)
    idx_flat = np.swapaxes(idx_flat, 2, 3)                 # [core,Bc,par,pair,P]
    idx_flat = idx_flat.reshape(N_CORES, Bc, 2, PAIRS_PER_BLOCK * P).astype(np.int16)
    GIDX = PAIRS_PER_BLOCK * P // 16
    wrapped = idx_flat.reshape(N_CORES, Bc, 2, GIDX, 16)   # [.., col, row16]
    wrapped = np.swapaxes(wrapped, 3, 4)                   # [.., row16, col]
    idxw = np.broadcast_to(wrapped[:, :, :, None, :, :],
                           (N_CORES, Bc, 2, 8, 16, GIDX))
    idxw = np.ascontiguousarray(idxw.reshape(N_CORES, Bc, 2, P, GIDX))

    in_maps = []
    for m in range(N_CORES):
        in_maps.append({
            "fe": fe,
            "fo": fo,
            "idxe": idxw[m, :, 0],
            "idxo": idxw[m, :, 1],
            "shw": shw_h[m],
            "dstw": dstw_h[m],
            "iot": iot_host,
            "tmat": tmat_host,
        })
    return Bc, in_maps, win_ids


def assemble(dev_rows, win_ids):
    # device row ((m*Bc + b)*8 + pair)*16 + slot  == win_ids[global_pair, slot]
    ids_flat = win_ids.reshape(-1)
    mask = ids_flat >= 0
    out_full = np.zeros((NUM_NODES, FDIM), dtype=np.float32)
    out_full[ids_flat[mask]] = dev_rows[mask]
    return out_full


def kernel_with_result(features, edge_sh, edge_index, weights, trace=False):
    Bc, in_maps, win_ids = prepare(features, edge_sh, edge_index, weights)
    nc = build_program(Bc)
    res = run_bass_kernel_spmd(nc, in_maps, core_ids=list(range(N_CORES)), trace=trace)
    dev_rows = np.concatenate([res.results[m]["out"] for m in range(N_CORES)], axis=0)
    return assemble(dev_rows, win_ids), res


def kernel(features, edge_sh, edge_index, weights):
    """Harness entry point: full inputs in, full [50000, 128] output back."""
    out, _ = kernel_with_result(features, edge_sh, edge_index, weights, trace=False)
    return out
